# revision 29
# baseline (speedup 1.0000x reference)
"""MoE layer (top-2 of 8 experts) on 8 TRN2 NeuronCores, expert-parallel.

Host side: router (exact replica of the reference jax ops, so top-k
selection bit-matches), token gather by expert assignment, weight
repacking into DMA-friendly layouts + cast to bf16, and the final
weighted scatter-add.

Device side (one expert per core, SPMD): the full expert FFN
    h = X @ W1 ; act = gelu(h_gate) * h_up ; Y = act @ W2
matmuls run in bf16 (full PE rate, fast weight load, ~3e-3 rel err)
accumulating in fp32 PSUM, with all activations kept transposed
(tokens on the free axis) so no on-device transposes are needed.
Token chunks iterate *inside* the k-accumulation loop so one weight
load feeds all chunks and the PE streams wall-to-wall.

Load balancing: per-core capacity is capped at C=1024 (= N*K/E, the
mean pairs/expert). Experts are near-uniform here (router logits are
tiny), so only the small overflow above the mean (<1% of pairs) is
computed on host in exact fp32 — this equalizes the per-core device
work instead of padding every core to the hottest expert's count.

Self-contained: only library imports (numpy/jax/ml_dtypes/concourse).
"""

import numpy as np
import ml_dtypes

BF16 = ml_dtypes.bfloat16

TOP_K = 2
EPS = 1e-6
P = 128
D = 2048
F = 2048  # expert hidden dim (ED)
E = 8
KO = D // P  # 16 K-tiles for matmul1 / output D-tiles
MJ = F // P  # 16 gate/up tile pairs; also K-tiles for matmul2
CCAP = 1024  # per-core token capacity (mean pairs/expert); excess -> host

_BUILD_CACHE: dict = {}

# Activation for the gate branch. CoreSim doesn't implement Gelu, so tests
# can set this to "Identity" for structural sim validation.
ACT_FN = "Gelu"


def _chunks_of(C: int) -> list[tuple[int, int]]:
    """Split the token-capacity free axis into equal matmul chunks <= 512.

    At least 3 chunks: the PE only reaches stream rate with >=3
    interleaved accumulation groups in flight (2-chunk interleave is
    duration-limited at ~1.21 cycles/col vs ~1.03 measured with 3).
    """
    if C <= 512:
        return [(0, C)]
    nch = max(-(-C // 512), 3)
    base = C // nch
    base -= base % 8
    sizes = [base] * nch
    rem = C - base * nch
    i = 0
    while rem > 0:
        add = min(8, rem)
        sizes[i % nch] += add
        rem -= add
        i += 1
    out = []
    off = 0
    for s in sizes:
        out.append((off, s))
        off += s
    assert off == C
    return out


def _build(C: int):
    """Build + compile the per-core expert-FFN bass program for capacity C."""
    key = (C, ACT_FN)
    if key in _BUILD_CACHE:
        return _BUILD_CACHE[key]

    import concourse.bacc as bacc
    import concourse.mybir as mybir
    import concourse.tile as tile
    f32 = mybir.dt.float32
    bf16 = mybir.dt.bfloat16
    act_fn = getattr(mybir.ActivationFunctionType, ACT_FN)
    chunks = _chunks_of(C)
    NCH = len(chunks)

    nc = bacc.Bacc(
        "TRN2", target_bir_lowering=False, debug=False, enable_asserts=False
    )
    # Packed layouts (host pre-transposed, partition-major, bf16):
    #   xt[p, ko, c]    = X^T[ko*128+p, c]          (tokens on free axis)
    #   w1[p, m, ko, q] = W1perm[ko*128+p, m*128+q] (m: g0,u0,g1,u1,... strips)
    #   w2[p, i, fo, q] = W2[fo*128+p, i*128+q]
    #   yt[p, io, c]    = Y^T[io*128+p, c]
    xt_d = nc.dram_tensor("xt", [P, KO, C], bf16, kind="ExternalInput")
    w1_d = nc.dram_tensor("w1", [P, 2 * MJ, KO, P], bf16, kind="ExternalInput")
    w2_d = nc.dram_tensor("w2", [P, KO, MJ, P], bf16, kind="ExternalInput")
    yt_d = nc.dram_tensor("yt", [P, KO, C], bf16, kind="ExternalOutput")

    with tile.TileContext(nc) as tc:
        with (
            tc.tile_pool(name="xt", bufs=1) as xt_pool,
            tc.tile_pool(name="act", bufs=1) as act_pool,
            tc.tile_pool(name="w1h", bufs=1) as w1h_pool,
            tc.tile_pool(name="w1", bufs=6) as w1_pool,
            tc.tile_pool(name="w2", bufs=3) as w2_pool,
            tc.tile_pool(name="tg", bufs=6) as tg_pool,
            tc.tile_pool(name="yo", bufs=4) as yo_pool,
            tc.tile_pool(name="warm", bufs=1) as warm_pool,
            tc.tile_pool(name="ps", bufs=8, space="PSUM") as ps_pool,
        ):
            # HAM warmup: a burst of throwaway matmuls on a zeroed scratch
            # tile keeps the PE busy from the end of the preamble, so the
            # clock gate is already at 8/8 when the first real matmul's
            # input lands (~4us of streaming activity flips it).
            warm_sb = warm_pool.tile([P, P], bf16)
            nc.vector.memset(warm_sb[:], 0)
            warm_ps = ps_pool.tile([P, 512], f32, tag="ps", name="warm_ps")
            for _ in range(28):
                nc.tensor.matmul(warm_ps[:, :P], warm_sb[:], warm_sb[:])

            # DMA plan. Both HWDGE rings carry the opening set in PE
            # consumption order. j0+j1 run fused with an 8-way interleave
            # (g0,u0,g1,u1 x 2 chunks per k-step), so the PE consumes xt at
            # only ~150 GB/s during the fill phase — comfortably under the
            # two rings' supply — and the HAM clock gate never drops.
            # The 4 early strips (g0,u0,g1,u1) are adjacent in w1's m dim,
            # so one DMA per k-quarter covers all of them.
            w1_tiles = {}

            def issue_w1(m, ring=None):
                t = w1_pool.tile([P, KO, P], bf16, tag="w1s")
                (ring or nc.sync).dma_start(t[:], w1_d.ap()[:, m])
                w1_tiles[m] = t

            xt_sb = xt_pool.tile([P, KO, C], bf16)
            w1h = w1h_pool.tile([P, 4, KO, P], bf16)

            # Opening DMA, interleaved across both rings in PE consumption
            # order. j0+j1 run fused: first over chunks 0+1 together (8
            # PSUM banks), then chunk 2 — so the fill phase needs the 4
            # head strips (g0,u0,g1,u1 — adjacent in w1's m dim, one DMA
            # per k-range covers all) + the first chunks' xt columns, at a
            # demand rate two rings comfortably supply.
            cb = chunks[0][1] + chunks[1][1]
            nc.sync.dma_start(w1h[:, :, 0:2], w1_d.ap()[:, 0:4, 0:2])
            nc.scalar.dma_start(xt_sb[:, 0:2, :cb], xt_d.ap()[:, 0:2, :cb])
            nc.sync.dma_start(xt_sb[:, 2:4, :cb], xt_d.ap()[:, 2:4, :cb])
            nc.scalar.dma_start(w1h[:, :, 2:4], w1_d.ap()[:, 0:4, 2:4])
            nc.sync.dma_start(w1h[:, :, 4:8], w1_d.ap()[:, 0:4, 4:8])
            nc.scalar.dma_start(xt_sb[:, 4:6, :cb], xt_d.ap()[:, 4:6, :cb])
            nc.sync.dma_start(xt_sb[:, 6:8, :cb], xt_d.ap()[:, 6:8, :cb])
            nc.scalar.dma_start(xt_sb[:, 8:12, :cb], xt_d.ap()[:, 8:12, :cb])
            nc.sync.dma_start(w1h[:, :, 8:16], w1_d.ap()[:, 0:4, 8:16])
            nc.scalar.dma_start(xt_sb[:, 12:16, :cb], xt_d.ap()[:, 12:16, :cb])
            nc.sync.dma_start(xt_sb[:, 0:8, cb:], xt_d.ap()[:, 0:8, cb:])
            nc.scalar.dma_start(xt_sb[:, 8:16, cb:], xt_d.ap()[:, 8:16, cb:])
            # j2..j4 strip prefetch behind the critical opening set
            # (exactly fills the 6-slot w1 pool). m=4 (j2's gate strip)
            # rides the lighter scalar ring so it lands well before j2.
            issue_w1(4, nc.scalar)
            issue_w1(5, nc.sync)
            issue_w1(6, nc.scalar)
            issue_w1(7, nc.sync)
            issue_w1(8, nc.scalar)
            issue_w1(9, nc.sync)

            act_sb = act_pool.tile([P, MJ, C], bf16)

            def ps_tiles(pref):
                return [
                    ps_pool.tile([P, 512], f32, tag="ps", name=f"{pref}{ci}")
                    for ci in range(NCH)
                ]

            with nc.named_scope("ffn1"):
                # --- j0+j1 fused, phased over chunk groups: phase 1 runs
                # chunks 0+1 (4 accumulation groups x 2 chunks = all 8 PSUM
                # banks, 8 matmuls per xt piece -> lowest fill-phase DMA
                # demand), phase 2 runs chunk 2 (4 banks) while phase 1's
                # gelu/mul drain, and j2 then overlaps phase 2's drain. ---
                for phase in (chunks[:2], chunks[2:]):
                    pgu = {}
                    for jj in range(2):
                        for gu in range(2):
                            for ci in range(len(phase)):
                                pgu[jj, gu, ci] = ps_pool.tile(
                                    [P, 512], f32, tag="ps", name=f"ph{jj}{gu}{ci}"
                                )
                    for ko in range(KO):
                        for jj in range(2):
                            for gu in range(2):
                                for ci, (c0, cn) in enumerate(phase):
                                    nc.tensor.matmul(
                                        pgu[jj, gu, ci][:, :cn],
                                        w1h[:, 2 * jj + gu, ko],
                                        xt_sb[:, ko, c0 : c0 + cn],
                                        start=(ko == 0),
                                        stop=(ko == KO - 1),
                                    )
                    for jj in range(2):
                        for ci, (c0, cn) in enumerate(phase):
                            tg = tg_pool.tile(
                                [P, 512], f32, tag="tg", name=f"tgh{jj}{ci}"
                            )
                            nc.scalar.activation(
                                tg[:, :cn], pgu[jj, 0, ci][:, :cn], act_fn
                            )
                            nc.vector.tensor_mul(
                                out=act_sb[:, jj, c0 : c0 + cn],
                                in0=tg[:, :cn],
                                in1=pgu[jj, 1, ci][:, :cn],
                            )

                # --- j2..j15: standard pipeline, prefetch 3 pairs ahead ---
                for j in range(2, MJ):
                    # prefetch strips three j-pairs ahead (m=4..9 came in
                    # the opening set); sync ring only, so the scalar (ACT)
                    # queue never blocks on a strip-slot wait ahead of gelu
                    if 2 * j + 6 < 2 * MJ:
                        issue_w1(2 * j + 6, nc.sync)
                    if 2 * j + 7 < 2 * MJ:
                        issue_w1(2 * j + 7, nc.sync)
                    wg = w1_tiles.pop(2 * j)
                    wu = w1_tiles.pop(2 * j + 1)
                    pg = ps_tiles(f"pg{j}")
                    for ko in range(KO):
                        for ci, (c0, cn) in enumerate(chunks):
                            nc.tensor.matmul(
                                pg[ci][:, :cn],
                                wg[:, ko],
                                xt_sb[:, ko, c0 : c0 + cn],
                                start=(ko == 0),
                                stop=(ko == KO - 1),
                            )
                    # gelu(gate) on ScalarE overlaps the up-projection matmuls
                    tg = [
                        tg_pool.tile([P, 512], f32, tag="tg", name=f"tg{ci}")
                        for ci in range(NCH)
                    ]
                    for ci, (c0, cn) in enumerate(chunks):
                        nc.scalar.activation(tg[ci][:, :cn], pg[ci][:, :cn], act_fn)
                    pu = ps_tiles(f"pu{j}")
                    for ko in range(KO):
                        for ci, (c0, cn) in enumerate(chunks):
                            nc.tensor.matmul(
                                pu[ci][:, :cn],
                                wu[:, ko],
                                xt_sb[:, ko, c0 : c0 + cn],
                                start=(ko == 0),
                                stop=(ko == KO - 1),
                            )
                    for ci, (c0, cn) in enumerate(chunks):
                        nc.vector.tensor_mul(
                            out=act_sb[:, j, c0 : c0 + cn],
                            in0=tg[ci][:, :cn],
                            in1=pu[ci][:, :cn],
                        )

            with nc.named_scope("ffn2"):
                w2_tiles = {}

                def issue_w2(i):
                    t = w2_pool.tile([P, MJ, P], bf16, tag="w2s")
                    nc.sync.dma_start(t[:], w2_d.ap()[:, i])
                    w2_tiles[i] = t

                issue_w2(0)
                issue_w2(1)
                # last tile's chunk list: final chunk halved, so the
                # end-of-kernel copy+store+receipt chain is half as long
                lc0, lcn = chunks[-1]
                h1 = (lcn // 2 + 7) & ~7
                tail_chunks = list(chunks[:-1]) + [(lc0, h1), (lc0 + h1, lcn - h1)]
                for i in range(KO):
                    if i + 2 < KO:
                        issue_w2(i + 2)
                    w2t = w2_tiles.pop(i)
                    cl = tail_chunks if i == KO - 1 else chunks
                    py = [
                        ps_pool.tile([P, 512], f32, tag="ps", name=f"py{i}_{ci}")
                        for ci in range(len(cl))
                    ]
                    if i == KO - 1:
                        # last output tile: run chunks sequentially so the
                        # earlier chunks' stores overlap the later chunks'
                        # matmuls, shrinking the post-matmul tail
                        for ci, (c0, cn) in enumerate(cl):
                            for fo in range(MJ):
                                nc.tensor.matmul(
                                    py[ci][:, :cn],
                                    w2t[:, fo],
                                    act_sb[:, fo, c0 : c0 + cn],
                                    start=(fo == 0),
                                    stop=(fo == MJ - 1),
                                )
                    else:
                        for fo in range(MJ):
                            for ci, (c0, cn) in enumerate(cl):
                                nc.tensor.matmul(
                                    py[ci][:, :cn],
                                    w2t[:, fo],
                                    act_sb[:, fo, c0 : c0 + cn],
                                    start=(fo == 0),
                                    stop=(fo == MJ - 1),
                                )
                    for ci, (c0, cn) in enumerate(cl):
                        # bf16 output: halves the store traffic and the
                        # final-store tail; the host combine has ~5x error
                        # margin to spare.
                        yo = yo_pool.tile([P, 512], bf16, tag="yo")
                        if ci % 2 == 1:
                            nc.scalar.copy(out=yo[:, :cn], in_=py[ci][:, :cn])
                            ring = nc.sync
                        else:
                            nc.vector.tensor_copy(out=yo[:, :cn], in_=py[ci][:, :cn])
                            ring = nc.scalar
                        if i == KO - 1:
                            # tail: halve the final stores across both rings
                            # so the completion receipts overlap
                            h = cn // 2
                            nc.scalar.dma_start(
                                yt_d.ap()[:, i, c0 : c0 + h], yo[:, :h]
                            )
                            nc.sync.dma_start(
                                yt_d.ap()[:, i, c0 + h : c0 + cn], yo[:, h:cn]
                            )
                        else:
                            ring.dma_start(yt_d.ap()[:, i, c0 : c0 + cn], yo[:, :cn])

    nc.compile()
    _BUILD_CACHE[key] = nc
    return nc


def _router(x, router_scale, gate_w):
    """Replicate the reference router ops exactly (same jax ops, default
    backend) so the top-2 expert selection bit-matches the reference."""
    import jax
    import jax.numpy as jnp

    x = jnp.asarray(x)
    router_scale = jnp.asarray(router_scale)
    gate_w = jnp.asarray(gate_w)
    _B, _L, d = x.shape
    h = x * jax.lax.rsqrt(jnp.mean(x * x, axis=-1, keepdims=True) + EPS)
    h = h * (d**-0.5) * router_scale
    logits = (h @ gate_w).astype(jnp.float32)
    probs = jax.nn.softmax(logits, axis=-1)
    w, idx = jax.lax.top_k(probs, TOP_K)
    w = w / jnp.clip(jnp.sum(w, axis=-1, keepdims=True), 1e-12)
    w = w.astype(x.dtype)
    return (
        np.asarray(idx).reshape(-1, TOP_K),
        np.asarray(w).reshape(-1, TOP_K).astype(np.float32),
    )


def _pack_w1(gate_up_e: np.ndarray) -> np.ndarray:
    """[D, 2F] -> [P, 2*MJ, KO, P] bf16, gate/up 128-col strips interleaved."""
    g = gate_up_e[:, :F].reshape(D, MJ, P)
    u = gate_up_e[:, F:].reshape(D, MJ, P)
    w1p = np.empty((D, 2 * MJ, P), np.float32)
    w1p[:, 0::2] = g
    w1p[:, 1::2] = u
    # [D, 2MJ, P] -> [KO, P, 2MJ, P] -> [P, 2MJ, KO, P]
    return np.ascontiguousarray(
        w1p.reshape(KO, P, 2 * MJ, P).transpose(1, 2, 0, 3)
    ).astype(BF16)


def _pack_w2(down_e: np.ndarray) -> np.ndarray:
    """[F, D] -> [P, KO, MJ, P] bf16 (w2[p,i,fo,q] = W2[fo*128+p, i*128+q])."""
    return np.ascontiguousarray(
        down_e.reshape(MJ, P, KO, P).transpose(1, 2, 0, 3)
    ).astype(BF16)


def _host_ffn(x_rows: np.ndarray, gate_up_e: np.ndarray, down_e: np.ndarray):
    """Exact fp32 expert FFN for the few load-balancing overflow rows."""
    import jax
    import jax.numpy as jnp

    h = jnp.asarray(x_rows) @ jnp.asarray(gate_up_e)
    gate, up = h[:, :F], h[:, F:]
    act = jax.nn.gelu(gate, approximate=False) * up
    return np.asarray(act @ jnp.asarray(down_e), dtype=np.float32)


def run_moe(x, router_scale, gate_w, gate_up, down, per_expert_scale, trace=False):
    from concourse import bass_utils

    x = np.asarray(x, dtype=np.float32)
    router_scale = np.asarray(router_scale, dtype=np.float32)
    gate_w = np.asarray(gate_w, dtype=np.float32)
    gate_up = np.asarray(gate_up, dtype=np.float32)
    down = np.asarray(down, dtype=np.float32)
    per_expert_scale = np.asarray(per_expert_scale, dtype=np.float32)

    B, L, d = x.shape
    N = B * L
    assert d == D and gate_up.shape == (E, D, 2 * F) and down.shape == (E, F, D)

    idxf, wf = _router(x, router_scale, gate_w)

    pair_expert = idxf.reshape(-1)
    pair_token = np.repeat(np.arange(N), TOP_K)
    pair_w = wf.reshape(-1) * per_expert_scale[pair_expert]

    order = np.argsort(pair_expert, kind="stable")
    tok_o = pair_token[order]
    w_o = pair_w[order]
    counts = np.bincount(pair_expert, minlength=E)
    offs = np.zeros(E + 1, np.int64)
    offs[1:] = np.cumsum(counts)

    # Device capacity = mean pairs/expert; per-expert overflow above the
    # mean (<1% of pairs for this near-uniform router) is handled on host.
    C = CCAP

    nc = _build(C)

    xf = x.reshape(N, D)
    w1_packed = [_pack_w1(gate_up[e]) for e in range(E)]
    w2_packed = [_pack_w2(down[e]) for e in range(E)]

    contrib = np.empty((len(tok_o), D), np.float32)
    in_maps = []
    ranges = []
    host_ranges = []
    for e in range(E):
        lo = offs[e]
        hi = min(lo + C, offs[e + 1])
        toks = tok_o[lo:hi]
        ranges.append((lo, hi))
        if hi < offs[e + 1]:
            host_ranges.append((e, hi, offs[e + 1]))
        xg = np.zeros((C, D), np.float32)
        xg[: len(toks)] = xf[toks]
        xt = np.ascontiguousarray(
            xg.T.reshape(KO, P, C).transpose(1, 0, 2)
        ).astype(BF16)
        in_maps.append({"xt": xt, "w1": w1_packed[e], "w2": w2_packed[e]})

    res = bass_utils.run_bass_kernel_spmd(
        nc, in_maps, core_ids=list(range(E)), trace=trace
    )
    for e in range(E):
        lo, hi = ranges[e]
        yt = np.asarray(res.results[e]["yt"]).astype(np.float32)  # [P, KO, C]
        ytd = yt.transpose(1, 0, 2).reshape(D, C)  # [D, C]
        contrib[lo:hi] = ytd[:, : hi - lo].T

    # Host-side exact FFN for the overflow rows (load balancing remainder).
    for e, lo, hi in host_ranges:
        contrib[lo:hi] = _host_ffn(xf[tok_o[lo:hi]], gate_up[e], down[e])

    contrib *= w_o[:, None]

    s = np.argsort(tok_o, kind="stable")
    tok_s = tok_o[s]
    out = np.zeros((N, D), np.float32)
    if len(tok_s) == 2 * N and np.array_equal(tok_s[0::2], tok_s[1::2]):
        cs = contrib[s]
        out[tok_s[0::2]] = cs[0::2] + cs[1::2]
    else:  # defensive fallback (duplicate experts per token can't happen)
        np.add.at(out, tok_o, contrib)
    return out.reshape(B, L, D), res


def kernel(x, router_scale, gate_w, gate_up, down, per_expert_scale):
    out, _ = run_moe(x, router_scale, gate_w, gate_up, down, per_expert_scale)
    return out
